# revision 48
# baseline (speedup 1.0000x reference)
"""Trainium2 Bass kernel for a 12-head attention block.

Problem (hardcoded): x [16, 1024, 768] f32, w_qkv [2304, 768], w_proj
[768, 768], b_proj [768].  out = proj(softmax(q k^T / sqrt(64)) v).

Sharding: pure data parallel over batch - 16 batches / 8 cores = 2
batches per core, no collectives.  All layout transposes happen on the
host: each core receives x^T slabs and produces out^T slabs.

Design (vs the fp32r baseline, 485us -> ~340us):
  * all matmul operands in bf16 (PSUM accumulation stays fp32): enables
    fast-weight-load and halves SBUF/DMA traffic; rel-err ~4e-3 vs the
    2e-2 budget.
  * QK^T processes HEAD PAIRS with PE row tiling: even head at array
    rows 0-63 (tile_position (0,0)), odd head at rows 64-127 ((64,0)).
    The two K=64 matmuls run concurrently -> ~2x effective throughput
    (measured 122ns avg per 512-row matmul vs 213ns streaming floor).
    qt/kt are laid out head-major so the partition ranges line up.
  * exp on ACT covers a head pair per instruction: S psum tile is
    [128, 1024] = [j-chunk, (even 512 | odd 512)] spanning two banks,
    amortizing the ~300-cycle ACT instruction overhead.
  * qkv projection of batch b+1 / output projection of batch b-1 are
    interleaved INSIDE the attention jc-loops (between QK^T, which
    never waits on the current exp, and PV, which does) with
    deadline-aware pacing, filling PE idle while ACT grinds exps.
  * cross-block software pipeline: the next block's first QK^T pair
    issues before the current block's last PVs, and each head's
    normalize chain is emitted right after its last PV so the O psum
    bank releases as early as possible.
  * V stays ones-augmented ([128, 65] stationary) so the softmax
    denominator falls out of the PV matmul as row 64 (col-pairing two
    heads at M=64 each is a wash: the separate denominator matmuls cost
    exactly the column waste back).
  * PSUM budget (8 banks): S 2x[128,1024]=4, O 2x[65,512]=2, qkv/proj
    2x[128,512]=2.  Single-bank general pool or 3-deep O both lose
    (head-of-line serialization of the in-order PE queue).
"""

import numpy as np
from contextlib import ExitStack

import concourse.bass as bass
import concourse.mybir as mybir
import concourse.tile as tile
from concourse import bacc
from concourse import bass_utils

F32 = mybir.dt.float32
BF16 = mybir.dt.bfloat16
EXP = mybir.ActivationFunctionType.Exp

B, N, C = 16, 1024, 768
H, D = 12, 64
E = 3 * C
NCORES = 8
BL = B // NCORES          # batches per core
T = BL * N                # tokens per core
KC = C // 128             # feature chunks of 128
JC = N // 128             # token chunks of 128
NP = H // 2               # head pairs
SCALE = float(D) ** -0.5

_CACHE = {}


def _mm(nc, out, lhsT, rhs, **kw):
    nc.tensor.matmul(out, lhsT=lhsT, rhs=rhs, **kw)


def _build(ctx, tc):
    nc = tc.nc
    dram = ctx.enter_context(tc.tile_pool(name="dram", bufs=1, space="DRAM"))
    # x^T blocked: [kc, b, 128, N] so each per-batch chunk is one contiguous slab
    xT_d = dram.tile([KC, BL, 128, N], BF16, kind="ExternalInput", name="xTb", uniquify=False)
    # w_qkv^T as per-kc slabs [kc, 128, 2304]
    wqkv_d = dram.tile([KC, 128, E], BF16, kind="ExternalInput", name="wqkvb", uniquify=False)
    # w_proj^T per-kc slabs [kc, 128, 768]
    wproj_d = dram.tile([KC, 128, C], BF16, kind="ExternalInput", name="wprojb", uniquify=False)
    bproj_d = dram.tile([C, 1], F32, kind="ExternalInput", name="bproj", uniquify=False)
    # out^T blocked: [oc, b, 128, N]
    outT_d = dram.tile([KC, BL, 128, N], F32, kind="ExternalOutput", name="outTb", uniquify=False)

    consts = ctx.enter_context(tc.tile_pool(name="consts", bufs=1))
    wp_pool = ctx.enter_context(tc.tile_pool(name="wproj", bufs=KC))
    wqk_pool = ctx.enter_context(tc.tile_pool(name="wqk", bufs=KC))
    wv_pool = ctx.enter_context(tc.tile_pool(name="wv", bufs=KC))
    xo_pool = ctx.enter_context(tc.tile_pool(name="xo", bufs=2 * KC))
    qk_pool = ctx.enter_context(tc.tile_pool(name="qkpool", bufs=4))
    va_pool = ctx.enter_context(tc.tile_pool(name="vpool", bufs=2 * JC))
    ot_pool = ctx.enter_context(tc.tile_pool(name="otpool", bufs=2 * KC))
    pt_pool = ctx.enter_context(tc.tile_pool(name="ppool", bufs=3))
    sm_pool = ctx.enter_context(tc.tile_pool(name="small", bufs=6))
    lb_pool = ctx.enter_context(tc.tile_pool(name="lbpool", bufs=4))
    # PSUM: 8 banks total.  s-tiles 2x[128,1024] = 4 banks, O 2x[65,512]
    # = 2 banks, general (qkv/proj) 2x[128,512] = 2 banks.
    ps_s = ctx.enter_context(tc.tile_pool(name="pss", bufs=2, space="PSUM"))
    ps_o = ctx.enter_context(tc.tile_pool(name="pso", bufs=2, space="PSUM"))
    ps_gp = ctx.enter_context(tc.tile_pool(name="psgp", bufs=2, space="PSUM"))

    vones = consts.tile([128, H, 1], BF16)
    nc.vector.memset(vones, 1.0)
    bias_sb = consts.tile([128, KC], F32)
    nc.sync.dma_start(
        out=bias_sb, in_=bproj_d[:, 0].rearrange("(k p) -> p k", p=128)
    )

    # batch-0 x first: the very first matmuls wait on these, so their DMAs
    # must be at the head of the queues, ahead of the bulk weight preload.
    # large transfers are split into column chunks so they spread across
    # DMA queues (a single queue moves only ~22 GB/s).
    xt = {}
    for kc in range(KC):
        xtc = xo_pool.tile([128, N], BF16, name=f"xt0_{kc}", tag="xo")
        nc.sync.dma_start(out=xtc, in_=xT_d[kc, 0])
        xt[(0, kc)] = xtc
    # q/k weights resident: per-kc [128, 12, 128] tile
    wqk_t = []
    for kc in range(KC):
        t = wqk_pool.tile([128, 2 * KC, 128], BF16, name=f"wqk{kc}", tag="wqk")
        nc.sync.dma_start(out=t.rearrange("p a b -> p (a b)"), in_=wqkv_d[kc, :, 0:2 * C])
        wqk_t.append(t)
    wqk = {(j, kc): wqk_t[kc][:, j, :] for j in range(2 * KC) for kc in range(KC)}
    # v weights per-kc [128, 6, 128]
    wv = []
    for kc in range(KC):
        wvt = wv_pool.tile([128, KC, 128], BF16, name=f"wv{kc}", tag="wv")
        nc.sync.dma_start(out=wvt.rearrange("p a b -> p (a b)"), in_=wqkv_d[kc, :, 2 * C:3 * C])
        wv.append(wvt)
    wp = {}

    qt = {}
    kt = {}
    va = {}
    ot = {}

    # ---------- micro-op groups (emitted lazily, interleaved) ----------
    # Each group is (deadline, thunk); deadline (b, p) means the group
    # must be emitted before attention block (b, p, *) is emitted.

    FAR = 999

    def xdma_thunk(b):
        def go():
            for kc in range(KC):
                xtc = xo_pool.tile([128, N], BF16, name=f"xt{b}_{kc}", tag="xo")
                nc.sync.dma_start(out=xtc, in_=xT_d[kc, b])
                xt[(b, kc)] = xtc
        return go

    def wp_thunk():
        def go():
            for kc in range(KC):
                t = wp_pool.tile([128, KC, 128], BF16, name=f"wpk{kc}", tag="wp")
                nc.sync.dma_start(out=t.rearrange("p a b -> p (a b)"), in_=wproj_d[kc])
                for oc in range(KC):
                    wp[(kc, oc)] = t[:, oc, :]
        return go

    def qk_alloc(b):
        qt[b] = qk_pool.tile([128, KC, N], BF16, name=f"qt{b}", tag="qk")
        kt[b] = qk_pool.tile([128, KC, N], BF16, name=f"kt{b}", tag="qk")

    def qk_thunk(b, which, mt, hf):
        # one [128, 512] half-slab of q^T or k^T (head pair mt)
        def go():
            dest = qt[b] if which == 0 else kt[b]
            ps = ps_gp.tile([128, 512], F32, name=f"psqk{b}_{which}_{mt}_{hf}", tag="gp")
            for kc in range(KC):
                w = wqk[(which * KC + mt, kc)]
                _mm(nc, ps, w, xt[(b, kc)][:, hf * 512:(hf + 1) * 512],
                    start=(kc == 0), stop=(kc == KC - 1))
            nc.vector.tensor_copy(out=dest[:, mt, hf * 512:(hf + 1) * 512], in_=ps)
        return go

    def v_thunk(b, jc, half):
        # V in natural [j, d] layout for token chunk jc, ones-augmented.
        # half 0: heads 0-7 (512 cols); half 1: heads 8-11 (256 cols) + ones.
        def go():
            if half == 0:
                vat = va_pool.tile([128, H, D + 1], BF16, name=f"va{b}_{jc}", tag="va")
                va[(b, jc)] = vat
            else:
                vat = va[(b, jc)]
            vps = ps_gp.tile([128, 512], F32, name=f"vps{half}_{b}_{jc}", tag="gp")
            lo, sz = (0, 512) if half == 0 else (512, 256)
            for kc in range(KC):
                xs = xt[(b, kc)][:, jc * 128:(jc + 1) * 128]
                wvf = wv[kc].rearrange("p a b -> p (a b)")
                _mm(nc, vps[:, 0:sz], xs, wvf[:, lo:lo + sz],
                    start=(kc == 0), stop=(kc == KC - 1))
            if half == 0:
                nc.vector.tensor_copy(
                    out=vat[:, 0:8, 0:D], in_=vps.rearrange("p (h d) -> p h d", h=8)
                )
            else:
                nc.vector.tensor_copy(
                    out=vat[:, 8:12, 0:D],
                    in_=vps[:, 0:256].rearrange("p (h d) -> p h d", h=4),
                )
                nc.vector.tensor_copy(out=vat[:, :, D:D + 1], in_=vones)
        return go

    def proj_thunk(b, oc, hf):
        def go():
            pps = ps_gp.tile([128, 512], F32, name=f"pps{b}_{oc}_{hf}", tag="gp")
            for kc in range(KC):
                _mm(nc, pps, wp[(kc, oc)],
                    ot[(b, kc)][:, hf * 512:(hf + 1) * 512],
                    start=(kc == 0), stop=(kc == KC - 1))
            ob = sm_pool.tile([128, 512], F32, name=f"ob{b}_{oc}_{hf}", tag="sm")
            nc.vector.tensor_scalar_add(out=ob, in0=pps, scalar1=bias_sb[:, oc:oc + 1])
            nc.sync.dma_start(out=outT_d[oc, b, :, hf * 512:(hf + 1) * 512], in_=ob)
        return go

    # ---------- interleave queue ----------

    ops = []          # FIFO of (deadline_block_idx, thunk)
    state = {"slot": 0, "acc": 0.0, "total": 0}

    def drain_deadline(idx):
        while ops and ops[0][0] <= idx:
            ops.pop(0)[1]()

    def drain_paced():
        # deadline-aware pacing: emit fast enough that no deadline forces
        # a burst, but otherwise spread evenly over the remaining slots.
        # groups aim to land by the middle of the block BEFORE their
        # deadline so block boundaries stay clear of fill bursts.
        s = state["slot"]
        state["slot"] = s + 1
        rate = len(ops) / max(state["total"] - s, 1)
        cum = 0
        for dl, _ in ops[:24]:
            cum += 1
            if dl < FAR:
                rate = max(rate, cum / max((dl - 1) * JC + 4 - s, 1))
        state["acc"] += rate
        while state["acc"] >= 1.0 and ops:
            state["acc"] -= 1.0
            ops.pop(0)[1]()

    # ---------- attention for one (batch, pair, i-half) block ----------

    carry = {}

    def qkt_pair(b, p, hf, jc):
        i0 = hf * 512
        s = ps_s.tile([128, 1024], F32, name=f"s{b}_{p}_{hf}_{jc}", tag="s")
        _mm(nc, s[:, 0:512],
            kt[b][0:D, p, jc * 128:(jc + 1) * 128],
            qt[b][0:D, p, i0:i0 + 512])
        _mm(nc, s[:, 512:1024],
            kt[b][D:128, p, jc * 128:(jc + 1) * 128],
            qt[b][D:128, p, i0:i0 + 512])
        return s

    def attn_block(b, p, hf, nxt, nxt_bi):
        i0 = hf * 512
        o_e = ps_o.tile([D + 1, 512], F32, name=f"oe{b}_{p}_{hf}", tag="o")
        o_o = ps_o.tile([D + 1, 512], F32, name=f"oo{b}_{p}_{hf}", tag="o")

        s = carry.pop("s", None)
        if s is None:
            s = qkt_pair(b, p, hf, 0)
        for jc in range(JC):
            pt = pt_pool.tile([128, 1024], BF16, name=f"pt{b}_{p}_{hf}_{jc}", tag="pt")
            nc.scalar.activation(out=pt, in_=s, func=EXP, scale=SCALE)
            if jc + 1 < JC:
                s = qkt_pair(b, p, hf, jc + 1)
            elif nxt is not None:
                # cross-block software pipeline: next block's first QK^T
                # pair issues ahead of this block's last PVs so its exp
                # starts without an ACT gap at the boundary
                carry["s"] = qkt_pair(*nxt, 0)
            if jc == 4 and nxt is not None:
                drain_deadline(nxt_bi)
            # fill lands between QK^T (which never waits on this jc's exp)
            # and PV (which does) so the PE queue head never idles on ACT
            drain_paced()

            def norm(h01, o_ps):
                # rows 0..63 divided by l (= row 64)
                l_sb = sm_pool.tile([1, 512], F32, name=f"l{b}_{p}_{hf}_{h01}", tag="sm")
                nc.vector.tensor_copy(out=l_sb, in_=o_ps[D:D + 1, :])
                nc.vector.reciprocal_approx_fast(out=l_sb, in_=l_sb)
                lb = lb_pool.tile([D, 512], F32, name=f"lb{b}_{p}_{hf}_{h01}", tag="lb")
                nc.gpsimd.partition_broadcast(lb, l_sb, channels=D)
                nc.vector.tensor_mul(
                    out=ot[(b, p)][h01 * D:h01 * D + D, i0:i0 + 512],
                    in0=o_ps[0:D, :], in1=lb,
                )

            _mm(nc, o_e, va[(b, jc)][:, 2 * p, :], pt[:, 0:512],
                start=(jc == 0), stop=(jc == JC - 1))
            if jc == JC - 1:
                norm(0, o_e)
            _mm(nc, o_o, va[(b, jc)][:, 2 * p + 1, :], pt[:, 512:1024],
                start=(jc == 0), stop=(jc == JC - 1))
            if jc == JC - 1:
                norm(1, o_o)

    # ---------- program ----------

    for b in range(BL):
        for kc in range(KC):
            ot[(b, kc)] = ot_pool.tile([128, N], BF16, name=f"ot{b}_{kc}", tag="ot")

    # block schedule: batch 0 pair-major, batch 1 hf-major (so that all
    # hf=0 ot halves of batch 1 are done by block 18 and proj(1, hf=0)
    # can interleave into the last 6 blocks)
    blocks = [(0, p, hf) for p in range(NP) for hf in range(2)]
    blocks += [(1, p, 0) for p in range(NP)] + [(1, p, 1) for p in range(NP)]

    # batch-0 prologue: just enough for block (0,0,hf0) to start: k/q
    # slab-0 first halves (k chunks jc<4 live in the hf0 half) + first V
    qk_alloc(0)
    qk_thunk(0, 1, 0, 0)()
    qk_thunk(0, 0, 0, 0)()
    for half in range(2):
        v_thunk(0, 0, half)()

    # queue the rest; deadlines are block indices
    ops.append((1, qk_thunk(0, 1, 0, 1)))
    for half in range(2):
        ops.append((1, v_thunk(0, 1, half)))
    ops.append((1, qk_thunk(0, 0, 0, 1)))
    for jc in range(2, JC):
        for half in range(2):
            ops.append((1, v_thunk(0, jc, half)))
    for mt in range(1, KC):
        for which in range(2):
            for hf in range(2):
                ops.append((2 * mt, qk_thunk(0, which, mt, hf)))
    ops.append((10, xdma_thunk(1)))
    ops.append((12, wp_thunk()))
    qk_alloc(1)
    for jc in range(JC):
        for half in range(2):
            ops.append((12, v_thunk(1, jc, half)))
    for mt in range(KC):
        for which in range(2):
            for hf in range(2):
                ops.append((12 + mt, qk_thunk(1, which, mt, hf)))
    for oc in range(KC):
        for hf in range(2):
            ops.append((FAR, proj_thunk(0, oc, hf)))

    state["total"] = len(blocks) * JC
    for bi, (b, p, hf) in enumerate(blocks):
        drain_deadline(bi)
        nxt = blocks[bi + 1] if bi + 1 < len(blocks) else None
        attn_block(b, p, hf, nxt, bi + 1)
        if bi == 17:
            # batch-1 hf=0 ot halves complete: queue first-half proj(1)
            for oc in range(KC):
                ops.append((FAR, proj_thunk(1, oc, 0)))

    # anything left (stragglers), then tail: proj of batch 1 second half
    drain_deadline(FAR)
    for oc in range(KC):
        proj_thunk(1, oc, 1)()


def get_nc():
    if "nc" not in _CACHE:
        nc = bacc.Bacc(None, target_bir_lowering=False, debug=False)
        with tile.TileContext(nc) as tc:
            with ExitStack() as ctx:
                _build(ctx, tc)
        nc.compile()
        _CACHE["nc"] = nc
    return _CACHE["nc"]


def _to_bf16(a):
    import ml_dtypes
    return np.asarray(a, dtype=np.float32).astype(ml_dtypes.bfloat16)


def make_in_maps(x, w_qkv, w_proj, b_proj):
    x = np.asarray(x, dtype=np.float32)
    w_qkv = np.asarray(w_qkv, dtype=np.float32)
    w_proj = np.asarray(w_proj, dtype=np.float32)
    # w_qkv^T [c, e] -> per-kc slabs [kc, 128, 2304]
    wqkvb = _to_bf16(np.ascontiguousarray(w_qkv.T.reshape(KC, 128, E)))
    # w_proj^T [c, o] -> per-kc slabs [kc, 128, 768]
    wprojb = _to_bf16(np.ascontiguousarray(w_proj.T.reshape(KC, 128, C)))
    bp = np.ascontiguousarray(b_proj.astype(np.float32).reshape(C, 1))
    in_maps = []
    for c in range(NCORES):
        # x^T [c, t] -> blocks [kc, b, 128, N]
        xT = x[c * BL:(c + 1) * BL].reshape(T, C).T  # [768, 2048]
        xb = _to_bf16(np.ascontiguousarray(
            xT.reshape(KC, 128, BL, N).transpose(0, 2, 1, 3)
        ))
        in_maps.append({"xTb": xb, "wqkvb": wqkvb, "wprojb": wprojb, "bproj": bp})
    return in_maps


def assemble_out(results):
    outs = []
    for c in range(NCORES):
        ob = results[c]["outTb"]  # [oc, b, 128, N]
        oT = ob.transpose(0, 2, 1, 3).reshape(C, T)
        outs.append(np.ascontiguousarray(oT.T).reshape(BL, N, C))
    return np.concatenate(outs, axis=0).astype(np.float32)


def kernel(x, w_qkv, w_proj, b_proj):
    nc = get_nc()
    in_maps = make_in_maps(x, w_qkv, w_proj, b_proj)
    res = bass_utils.run_bass_kernel_spmd(nc, in_maps, core_ids=list(range(NCORES)))
    return assemble_out(res.results)


# revision 52
# speedup vs baseline: 1.0088x; 1.0088x over previous
"""Trainium2 Bass kernel for a 12-head attention block.

Problem (hardcoded): x [16, 1024, 768] f32, w_qkv [2304, 768], w_proj
[768, 768], b_proj [768].  out = proj(softmax(q k^T / sqrt(64)) v).

Sharding: pure data parallel over batch - 16 batches / 8 cores = 2
batches per core, no collectives.  All layout transposes happen on the
host: each core receives x^T slabs and produces out^T slabs.

Design (vs the fp32r baseline, 485us -> ~340us):
  * all matmul operands in bf16 (PSUM accumulation stays fp32): enables
    fast-weight-load and halves SBUF/DMA traffic; rel-err ~4e-3 vs the
    2e-2 budget.
  * QK^T processes HEAD PAIRS with PE row tiling: even head at array
    rows 0-63 (tile_position (0,0)), odd head at rows 64-127 ((64,0)).
    The two K=64 matmuls run concurrently -> ~2x effective throughput
    (measured 122ns avg per 512-row matmul vs 213ns streaming floor).
    qt/kt are laid out head-major so the partition ranges line up.
  * exp on ACT covers a head pair per instruction: S psum tile is
    [128, 1024] = [j-chunk, (even 512 | odd 512)] spanning two banks,
    amortizing the ~300-cycle ACT instruction overhead.
  * qkv projection of batch b+1 / output projection of batch b-1 are
    interleaved INSIDE the attention jc-loops (between QK^T, which
    never waits on the current exp, and PV, which does) with
    deadline-aware pacing, filling PE idle while ACT grinds exps.
  * cross-block software pipeline: the next block's first QK^T pair
    issues before the current block's last PVs, and each head's
    normalize chain is emitted right after its last PV so the O psum
    bank releases as early as possible.
  * V stays ones-augmented ([128, 65] stationary) so the softmax
    denominator falls out of the PV matmul as row 64 (col-pairing two
    heads at M=64 each is a wash: the separate denominator matmuls cost
    exactly the column waste back).
  * PSUM budget (8 banks): S 2x[128,1024]=4, O 2x[65,512]=2, qkv/proj
    2x[128,512]=2.  Single-bank general pool or 3-deep O both lose
    (head-of-line serialization of the in-order PE queue).
"""

import numpy as np
from contextlib import ExitStack

import concourse.bass as bass
import concourse.mybir as mybir
import concourse.tile as tile
from concourse import bacc
from concourse import bass_utils

F32 = mybir.dt.float32
BF16 = mybir.dt.bfloat16
EXP = mybir.ActivationFunctionType.Exp

B, N, C = 16, 1024, 768
H, D = 12, 64
E = 3 * C
NCORES = 8
BL = B // NCORES          # batches per core
T = BL * N                # tokens per core
KC = C // 128             # feature chunks of 128
JC = N // 128             # token chunks of 128
NP = H // 2               # head pairs
SCALE = float(D) ** -0.5

_CACHE = {}


def _mm(nc, out, lhsT, rhs, **kw):
    nc.tensor.matmul(out, lhsT=lhsT, rhs=rhs, **kw)


def _build(ctx, tc):
    nc = tc.nc
    dram = ctx.enter_context(tc.tile_pool(name="dram", bufs=1, space="DRAM"))
    # x^T blocked: [kc, b, 128, N] so each per-batch chunk is one contiguous slab
    xT_d = dram.tile([KC, BL, 128, N], BF16, kind="ExternalInput", name="xTb", uniquify=False)
    # w_qkv^T as per-kc slabs [kc, 128, 2304]
    wqkv_d = dram.tile([KC, 128, E], BF16, kind="ExternalInput", name="wqkvb", uniquify=False)
    # w_proj^T per-kc slabs [kc, 128, 768]
    wproj_d = dram.tile([KC, 128, C], BF16, kind="ExternalInput", name="wprojb", uniquify=False)
    bproj_d = dram.tile([C, 1], F32, kind="ExternalInput", name="bproj", uniquify=False)
    # out^T blocked: [oc, b, 128, N]
    outT_d = dram.tile([KC, BL, 128, N], F32, kind="ExternalOutput", name="outTb", uniquify=False)

    consts = ctx.enter_context(tc.tile_pool(name="consts", bufs=1))
    wp_pool = ctx.enter_context(tc.tile_pool(name="wproj", bufs=KC))
    wqk_pool = ctx.enter_context(tc.tile_pool(name="wqk", bufs=KC))
    wv_pool = ctx.enter_context(tc.tile_pool(name="wv", bufs=KC))
    xo_pool = ctx.enter_context(tc.tile_pool(name="xo", bufs=2 * KC))
    qk_pool = ctx.enter_context(tc.tile_pool(name="qkpool", bufs=4))
    va_pool = ctx.enter_context(tc.tile_pool(name="vpool", bufs=2 * JC))
    ot_pool = ctx.enter_context(tc.tile_pool(name="otpool", bufs=2 * KC))
    pt_pool = ctx.enter_context(tc.tile_pool(name="ppool", bufs=3))
    sm_pool = ctx.enter_context(tc.tile_pool(name="small", bufs=6))
    lb_pool = ctx.enter_context(tc.tile_pool(name="lbpool", bufs=4))
    # PSUM: 8 banks total.  s-tiles 2x[128,1024] = 4 banks, O 2x[65,512]
    # = 2 banks, general (qkv/proj) 2x[128,512] = 2 banks.
    ps_s = ctx.enter_context(tc.tile_pool(name="pss", bufs=2, space="PSUM"))
    ps_o = ctx.enter_context(tc.tile_pool(name="pso", bufs=2, space="PSUM"))
    ps_gp = ctx.enter_context(tc.tile_pool(name="psgp", bufs=2, space="PSUM"))

    vones = consts.tile([128, H, 1], BF16)
    nc.vector.memset(vones, 1.0)
    bias_sb = consts.tile([128, KC], F32)
    nc.sync.dma_start(
        out=bias_sb, in_=bproj_d[:, 0].rearrange("(k p) -> p k", p=128)
    )

    # batch-0 x first: the very first matmuls wait on these, so their DMAs
    # must be at the head of the queues, ahead of the bulk weight preload.
    # large transfers are split into column chunks so they spread across
    # DMA queues (a single queue moves only ~22 GB/s).
    xt = {}
    for kc in range(KC):
        xtc = xo_pool.tile([128, N], BF16, name=f"xt0_{kc}", tag="xo")
        nc.sync.dma_start(out=xtc, in_=xT_d[kc, 0])
        xt[(0, kc)] = xtc
    # q/k weights resident: per-kc [128, 12, 128] tile
    wqk_t = []
    for kc in range(KC):
        t = wqk_pool.tile([128, 2 * KC, 128], BF16, name=f"wqk{kc}", tag="wqk")
        nc.sync.dma_start(out=t.rearrange("p a b -> p (a b)"), in_=wqkv_d[kc, :, 0:2 * C])
        wqk_t.append(t)
    wqk = {(j, kc): wqk_t[kc][:, j, :] for j in range(2 * KC) for kc in range(KC)}
    # v weights per-kc [128, 6, 128]
    wv = []
    for kc in range(KC):
        wvt = wv_pool.tile([128, KC, 128], BF16, name=f"wv{kc}", tag="wv")
        nc.sync.dma_start(out=wvt.rearrange("p a b -> p (a b)"), in_=wqkv_d[kc, :, 2 * C:3 * C])
        wv.append(wvt)
    wp = {}

    qt = {}
    kt = {}
    va = {}
    ot = {}

    # ---------- micro-op groups (emitted lazily, interleaved) ----------
    # Each group is (deadline, thunk); deadline (b, p) means the group
    # must be emitted before attention block (b, p, *) is emitted.

    FAR = 999

    def xdma_thunk(b):
        def go():
            for kc in range(KC):
                xtc = xo_pool.tile([128, N], BF16, name=f"xt{b}_{kc}", tag="xo")
                nc.sync.dma_start(out=xtc, in_=xT_d[kc, b])
                xt[(b, kc)] = xtc
        return go

    def wp_thunk():
        def go():
            for kc in range(KC):
                t = wp_pool.tile([128, KC, 128], BF16, name=f"wpk{kc}", tag="wp")
                nc.sync.dma_start(out=t.rearrange("p a b -> p (a b)"), in_=wproj_d[kc])
                for oc in range(KC):
                    wp[(kc, oc)] = t[:, oc, :]
        return go

    def qk_alloc(b):
        qt[b] = qk_pool.tile([128, KC, N], BF16, name=f"qt{b}", tag="qk")
        kt[b] = qk_pool.tile([128, KC, N], BF16, name=f"kt{b}", tag="qk")

    def qk_thunk(b, which, mt, hf):
        # one [128, 512] half-slab of q^T or k^T (head pair mt)
        def go():
            dest = qt[b] if which == 0 else kt[b]
            ps = ps_gp.tile([128, 512], F32, name=f"psqk{b}_{which}_{mt}_{hf}", tag="gp")
            for kc in range(KC):
                w = wqk[(which * KC + mt, kc)]
                _mm(nc, ps, w, xt[(b, kc)][:, hf * 512:(hf + 1) * 512],
                    start=(kc == 0), stop=(kc == KC - 1))
            nc.vector.tensor_copy(out=dest[:, mt, hf * 512:(hf + 1) * 512], in_=ps)
        return go

    def v_thunk(b, jc, half):
        # V in natural [j, d] layout for token chunk jc, ones-augmented.
        # half 0: heads 0-7 (512 cols); half 1: heads 8-11 (256 cols) + ones.
        def go():
            if half == 0:
                vat = va_pool.tile([128, H, D + 1], BF16, name=f"va{b}_{jc}", tag="va")
                va[(b, jc)] = vat
            else:
                vat = va[(b, jc)]
            vps = ps_gp.tile([128, 512], F32, name=f"vps{half}_{b}_{jc}", tag="gp")
            lo, sz = (0, 512) if half == 0 else (512, 256)
            for kc in range(KC):
                xs = xt[(b, kc)][:, jc * 128:(jc + 1) * 128]
                wvf = wv[kc].rearrange("p a b -> p (a b)")
                _mm(nc, vps[:, 0:sz], xs, wvf[:, lo:lo + sz],
                    start=(kc == 0), stop=(kc == KC - 1))
            if half == 0:
                nc.vector.tensor_copy(
                    out=vat[:, 0:8, 0:D], in_=vps.rearrange("p (h d) -> p h d", h=8)
                )
            else:
                nc.vector.tensor_copy(
                    out=vat[:, 8:12, 0:D],
                    in_=vps[:, 0:256].rearrange("p (h d) -> p h d", h=4),
                )
                nc.vector.tensor_copy(out=vat[:, :, D:D + 1], in_=vones)
        return go

    def proj_thunk(b, oc, hf):
        def go():
            pps = ps_gp.tile([128, 512], F32, name=f"pps{b}_{oc}_{hf}", tag="gp")
            for kc in range(KC):
                _mm(nc, pps, wp[(kc, oc)],
                    ot[(b, kc)][:, hf * 512:(hf + 1) * 512],
                    start=(kc == 0), stop=(kc == KC - 1))
            ob = sm_pool.tile([128, 512], F32, name=f"ob{b}_{oc}_{hf}", tag="sm")
            nc.vector.tensor_scalar_add(out=ob, in0=pps, scalar1=bias_sb[:, oc:oc + 1])
            nc.sync.dma_start(out=outT_d[oc, b, :, hf * 512:(hf + 1) * 512], in_=ob)
        return go

    # ---------- interleave queue ----------

    ops = []          # FIFO of (deadline_block_idx, thunk)
    state = {"slot": 0, "acc": 0.0, "total": 0}

    def drain_deadline(idx):
        while ops and ops[0][0] <= idx:
            ops.pop(0)[1]()

    def drain_paced():
        # deadline-aware pacing: emit fast enough that no deadline forces
        # a burst, but otherwise spread evenly over the remaining slots.
        # groups aim to land by the middle of the block BEFORE their
        # deadline so block boundaries stay clear of fill bursts.
        s = state["slot"]
        state["slot"] = s + 1
        rate = len(ops) / max(state["total"] - s, 1)
        cum = 0
        for dl, _ in ops[:24]:
            cum += 1
            if dl < FAR:
                rate = max(rate, cum / max((dl - 1) * JC + 4 - s, 1))
        state["acc"] += rate
        while state["acc"] >= 1.0 and ops:
            state["acc"] -= 1.0
            ops.pop(0)[1]()

    # ---------- attention for one (batch, pair, i-half) block ----------

    carry = {}

    def qkt_pair(b, p, hf, jc):
        i0 = hf * 512
        s = ps_s.tile([128, 1024], F32, name=f"s{b}_{p}_{hf}_{jc}", tag="s")
        _mm(nc, s[:, 0:512],
            kt[b][0:D, p, jc * 128:(jc + 1) * 128],
            qt[b][0:D, p, i0:i0 + 512])
        _mm(nc, s[:, 512:1024],
            kt[b][D:128, p, jc * 128:(jc + 1) * 128],
            qt[b][D:128, p, i0:i0 + 512])
        return s

    def attn_block(b, p, hf, nxt, nxt_bi):
        i0 = hf * 512
        o_e = ps_o.tile([D + 1, 512], F32, name=f"oe{b}_{p}_{hf}", tag="o")
        o_o = ps_o.tile([D + 1, 512], F32, name=f"oo{b}_{p}_{hf}", tag="o")

        s = carry.pop("s", None)
        if s is None:
            s = qkt_pair(b, p, hf, 0)
        for jc in range(JC):
            pt = pt_pool.tile([128, 1024], BF16, name=f"pt{b}_{p}_{hf}_{jc}", tag="pt")
            nc.scalar.activation(out=pt, in_=s, func=EXP, scale=SCALE)
            if jc + 1 < JC:
                s = qkt_pair(b, p, hf, jc + 1)
            elif nxt is not None:
                # cross-block software pipeline: next block's first QK^T
                # pair issues ahead of this block's last PVs so its exp
                # starts without an ACT gap at the boundary
                carry["s"] = qkt_pair(*nxt, 0)
            if jc == 4 and nxt is not None:
                drain_deadline(nxt_bi)
            # fill lands between QK^T (which never waits on this jc's exp)
            # and PV (which does) so the PE queue head never idles on ACT
            drain_paced()

            def norm(h01, o_ps):
                # rows 0..63 divided by l (= row 64)
                l_sb = sm_pool.tile([1, 512], F32, name=f"l{b}_{p}_{hf}_{h01}", tag="sm")
                nc.vector.tensor_copy(out=l_sb, in_=o_ps[D:D + 1, :])
                nc.vector.reciprocal_approx_fast(out=l_sb, in_=l_sb)
                lb = lb_pool.tile([D, 512], F32, name=f"lb{b}_{p}_{hf}_{h01}", tag="lb")
                nc.gpsimd.partition_broadcast(lb, l_sb, channels=D)
                nc.vector.tensor_mul(
                    out=ot[(b, p)][h01 * D:h01 * D + D, i0:i0 + 512],
                    in0=o_ps[0:D, :], in1=lb,
                )

            _mm(nc, o_e, va[(b, jc)][:, 2 * p, :], pt[:, 0:512],
                start=(jc == 0), stop=(jc == JC - 1))
            if jc == JC - 1:
                norm(0, o_e)
            _mm(nc, o_o, va[(b, jc)][:, 2 * p + 1, :], pt[:, 512:1024],
                start=(jc == 0), stop=(jc == JC - 1))
            if jc == JC - 1:
                norm(1, o_o)

    # ---------- program ----------

    for b in range(BL):
        for kc in range(KC):
            ot[(b, kc)] = ot_pool.tile([128, N], BF16, name=f"ot{b}_{kc}", tag="ot")

    # block schedule: batch 0 pair-major, batch 1 hf-major (so that all
    # hf=0 ot halves of batch 1 are done by block 18 and proj(1, hf=0)
    # can interleave into the last 6 blocks)
    blocks = [(0, p, hf) for p in range(NP) for hf in range(2)]
    blocks += [(1, p, 0) for p in range(NP)] + [(1, p, 1) for p in range(NP)]

    # batch-0 prologue: just enough for block (0,0,hf0) to start: k/q
    # slab-0 first halves (k chunks jc<4 live in the hf0 half) + first V
    qk_alloc(0)
    qk_thunk(0, 1, 0, 0)()
    qk_thunk(0, 0, 0, 0)()
    for half in range(2):
        v_thunk(0, 0, half)()

    # queue the rest; deadlines are block indices
    ops.append((1, qk_thunk(0, 1, 0, 1)))
    for half in range(2):
        ops.append((1, v_thunk(0, 1, half)))
    ops.append((1, qk_thunk(0, 0, 0, 1)))
    for jc in range(2, JC):
        for half in range(2):
            ops.append((1, v_thunk(0, jc, half)))
    for mt in range(1, KC):
        for which in range(2):
            for hf in range(2):
                ops.append((2 * mt, qk_thunk(0, which, mt, hf)))
    ops.append((10, xdma_thunk(1)))
    ops.append((12, wp_thunk()))
    qk_alloc(1)
    for jc in range(JC):
        for half in range(2):
            ops.append((12, v_thunk(1, jc, half)))
    for mt in range(KC):
        for which in range(2):
            for hf in range(2):
                ops.append((12 + mt, qk_thunk(1, which, mt, hf)))
    for oc in range(KC):
        for hf in range(2):
            ops.append((FAR, proj_thunk(0, oc, hf)))

    state["total"] = len(blocks) * JC
    for bi, (b, p, hf) in enumerate(blocks):
        drain_deadline(bi)
        nxt = blocks[bi + 1] if bi + 1 < len(blocks) else None
        attn_block(b, p, hf, nxt, bi + 1)
        if bi == 17:
            # batch-1 hf=0 ot halves complete: queue first-half proj(1)
            for oc in range(KC):
                ops.append((FAR, proj_thunk(1, oc, 0)))

    # anything left (stragglers), then tail: proj of batch 1 second half
    drain_deadline(FAR)
    for oc in range(KC):
        proj_thunk(1, oc, 1)()


def get_nc():
    if "nc" not in _CACHE:
        nc = bacc.Bacc(None, target_bir_lowering=False, debug=False)
        with tile.TileContext(nc) as tc:
            with ExitStack() as ctx:
                _build(ctx, tc)
        nc.compile()
        _CACHE["nc"] = nc
    return _CACHE["nc"]


def _to_bf16(a):
    import ml_dtypes
    return np.asarray(a, dtype=np.float32).astype(ml_dtypes.bfloat16)


def make_in_maps(x, w_qkv, w_proj, b_proj):
    x = np.asarray(x, dtype=np.float32)
    w_qkv = np.asarray(w_qkv, dtype=np.float32)
    w_proj = np.asarray(w_proj, dtype=np.float32)
    # w_qkv^T [c, e] -> per-kc slabs [kc, 128, 2304]
    wqkvb = _to_bf16(np.ascontiguousarray(w_qkv.T.reshape(KC, 128, E)))
    # w_proj^T [c, o] -> per-kc slabs [kc, 128, 768]
    wprojb = _to_bf16(np.ascontiguousarray(w_proj.T.reshape(KC, 128, C)))
    bp = np.ascontiguousarray(b_proj.astype(np.float32).reshape(C, 1))
    in_maps = []
    for c in range(NCORES):
        # x^T [c, t] -> blocks [kc, b, 128, N]
        xT = x[c * BL:(c + 1) * BL].reshape(T, C).T  # [768, 2048]
        xb = _to_bf16(np.ascontiguousarray(
            xT.reshape(KC, 128, BL, N).transpose(0, 2, 1, 3)
        ))
        in_maps.append({"xTb": xb, "wqkvb": wqkvb, "wprojb": wprojb, "bproj": bp})
    return in_maps


def assemble_out(results):
    outs = []
    for c in range(NCORES):
        ob = results[c]["outTb"]  # [oc, b, 128, N]
        oT = ob.transpose(0, 2, 1, 3).reshape(C, T)
        outs.append(np.ascontiguousarray(oT.T).reshape(BL, N, C))
    return np.concatenate(outs, axis=0).astype(np.float32)


def kernel(x, w_qkv, w_proj, b_proj):
    nc = get_nc()
    in_maps = make_in_maps(x, w_qkv, w_proj, b_proj)
    res = bass_utils.run_bass_kernel_spmd(nc, in_maps, core_ids=list(range(NCORES)))
    return assemble_out(res.results)
